# revision 1
# baseline (speedup 1.0000x reference)
"""AdapCNN block on 8 TRN2 NeuronCores.

Strategy (data-parallel over batch, 2 samples per core):
  - The tiny FMN weight-generator MLP (0.8% of FLOPs) runs on host in f32;
    the generated per-sample conv weights are sharded along B to the cores
    (exactly the "shard the generated per-sample weights along B" hint).
  - Each core runs the per-sample 64->64 3x3 VALID conv on its 2 samples.

Conv-as-matmul scheme (75% PE utilization):
  SBUF holds a row-pair duplicated copy of x: partitions 0:64 = channels at
  row r, partitions 64:128 = channels at row r+1.  One matmul per kw with
  lhsT[(t*64+c), (dq*64+o)] = W[o,c,t+dq,kw] * (0.5 if t+dq==1 else 1)
  computes, for PSUM slot j: partitions 0:64 += (kh0 + kh1/2) contribution of
  output row j, partitions 64:128 += (kh1/2 + kh2) contribution of output row
  j-1.  Output row q = psum[0:64, q] + psum[64:128, q+1] + bias, evicted with
  an ACT cross-partition copy + one DVE scalar_tensor_tensor.

Compute dtype bf16 (PSUM accumulates f32), output written f32.
"""
import sys

if '/opt/trn_rl_repo' not in sys.path:
    sys.path.insert(0, '/opt/trn_rl_repo')

import numpy as np
import ml_dtypes

B, CIN, COUT, K = 16, 64, 64, 3
H = W = 128
OH = OW = 126
FC, FMN0, FMN1, G = 512, 512, 512, 4
CNN_PARA = CIN * COUT * K * K + COUT
NCORES = 8
NS = B // NCORES          # samples per core
SB = 15                   # output rows per superblock (16 psum slots, overlap 1)
NSLOT = OH + 1            # 127 psum slots per sample

_cached = {}


def _build_module():
    import concourse.mybir as mybir
    import concourse.tile as tile
    from concourse import bacc

    f32 = mybir.dt.float32
    bf16 = mybir.dt.bfloat16

    nc = bacc.Bacc("TRN2", target_bir_lowering=False, debug=False,
                   num_devices=NCORES)
    x_ext = nc.declare_dram_parameter("xh", [NS, CIN, H, W], bf16,
                                      isOutput=False)
    wt_ext = nc.declare_dram_parameter("wt", [NS, 3, 128, 128], bf16,
                                       isOutput=False)
    b_ext = nc.declare_dram_parameter("bias", [COUT, NS], f32, isOutput=False)
    y_ext = nc.declare_dram_parameter("y", [NS, COUT, OH, OW], f32,
                                      isOutput=True)

    with tile.TileContext(nc) as tc:
        with tc.tile_pool(name="wts", bufs=1) as wpool, \
             tc.tile_pool(name="xin", bufs=2) as xpool, \
             tc.tile_pool(name="evac", bufs=3) as epool, \
             tc.tile_pool(name="outs", bufs=3) as opool, \
             tc.tile_pool(name="ps", bufs=2, space="PSUM") as pspool:

            wt_sb = wpool.tile([128, NS, 3, 128], bf16)
            for s in range(NS):
                for kw in range(3):
                    nc.sync.dma_start(wt_sb[:, s, kw, :], wt_ext[s, kw, :, :])
            bias_sb = wpool.tile([COUT, NS], f32)
            nc.sync.dma_start(bias_sb[:], b_ext[:])

            for s in range(NS):
                xp = xpool.tile([128, H, W], bf16, tag="xp")
                nc.sync.dma_start(xp[0:64, :, :], x_ext[s, :, :, :])
                nc.sync.dma_start(xp[64:128, 0:H - 1, :],
                                  x_ext[s, :, 1:H, :])

                nsb = (NSLOT + SB - 1) // SB  # 9 superblocks
                for bix in range(nsb):
                    j0 = SB * bix
                    j1 = min(j0 + SB + 1, NSLOT)
                    nslots = j1 - j0          # 16 (or 7 for the last)
                    nbanks = (nslots + 3) // 4
                    ps = pspool.tile([128, SB + 1, 128], f32, tag="ps")
                    for kw in range(3):
                        for kb in range(nbanks):
                            ss = 4 * kb
                            se = min(ss + 4, nslots)
                            nc.tensor.matmul(
                                ps[:, ss:se, 0:OW],
                                wt_sb[:, s, kw, :],
                                xp[:, j0 + ss:j0 + se, kw:kw + OW],
                                start=(kw == 0), stop=(kw == 2))
                    nrows = min(SB, OH - j0)
                    tmp = epool.tile([64, SB, OW], f32, tag="tmp")
                    nc.scalar.copy(tmp[:, 0:nrows, :],
                                   ps[64:128, 1:1 + nrows, 0:OW])
                    ob = opool.tile([64, SB, OW], f32, tag="ob")
                    nc.vector.scalar_tensor_tensor(
                        ob[:, 0:nrows, :],
                        ps[0:64, 0:nrows, 0:OW],
                        bias_sb[:, s:s + 1],
                        tmp[:, 0:nrows, :],
                        mybir.AluOpType.add,
                        mybir.AluOpType.add)
                    nc.sync.dma_start(y_ext[s, :, j0:j0 + nrows, :],
                                      ob[:, 0:nrows, :])
    nc.compile()
    return nc


def _fmn_host(fc_in, w1, b1, w2, b2, w3, b3):
    h = np.maximum(fc_in @ w1.T + b1, 0.0)
    h = np.maximum(h @ w2.T + b2, 0.0)
    hg = h.reshape(h.shape[0], G, FMN1 // G)
    o = np.einsum('bgi,goi->bgo', hg, w3,
                  dtype=np.float32).reshape(h.shape[0], -1) + b3
    return np.maximum(o, 0.0)


def kernel(x, fc_in, w1, b1, w2, b2, w3, b3, splits):
    from concourse.bass_utils import run_bass_kernel_spmd

    x = np.asarray(x, np.float32)
    fc_in = np.asarray(fc_in, np.float32)
    w1 = np.asarray(w1, np.float32)
    b1 = np.asarray(b1, np.float32)
    w2 = np.asarray(w2, np.float32)
    b2 = np.asarray(b2, np.float32)
    w3 = np.asarray(w3, np.float32)
    b3 = np.asarray(b3, np.float32)

    wb = _fmn_host(fc_in, w1, b1, w2, b2, w3, b3)          # [B, CNN_PARA]
    weight = wb[:, :-COUT].reshape(B, COUT, CIN, K, K)
    bias = wb[:, -COUT:]                                   # [B, COUT]

    # lhsT[s, kw, t*64+c, dq*64+o] = weight[s, o, c, t+dq, kw] * scale
    wk = weight.transpose(0, 4, 3, 2, 1)                   # [B, kw, kh, c, o]
    lhsT = np.empty((B, 3, 128, 128), np.float32)
    for t in (0, 1):
        for dq in (0, 1):
            kh = t + dq
            sc = 0.5 if kh == 1 else 1.0
            lhsT[:, :, t * 64:t * 64 + 64, dq * 64:dq * 64 + 64] = \
                wk[:, :, kh] * sc
    lhsT = lhsT.astype(ml_dtypes.bfloat16)

    xh = x.astype(ml_dtypes.bfloat16)

    if 'nc' not in _cached:
        _cached['nc'] = _build_module()
    nc = _cached['nc']

    in_maps = []
    for c in range(NCORES):
        s0 = NS * c
        in_maps.append({
            "xh": np.ascontiguousarray(xh[s0:s0 + NS]),
            "wt": np.ascontiguousarray(lhsT[s0:s0 + NS]),
            "bias": np.ascontiguousarray(bias[s0:s0 + NS].T),
        })

    res = run_bass_kernel_spmd(nc, in_maps, core_ids=list(range(NCORES)))

    out = np.empty((B * COUT, OH, OW), np.float32)
    for c in range(NCORES):
        y = res.results[c]["y"]                            # [NS, COUT, OH, OW]
        out[NS * COUT * c:NS * COUT * (c + 1)] = \
            np.asarray(y, np.float32).reshape(NS * COUT, OH, OW)
    return out.reshape(1, B * COUT, 1, OH, OW)
